# revision 18
# baseline (speedup 1.0000x reference)
"""Chamfer distance kernel for 8 Trainium2 NeuronCores.

Problem: preds [4, 8192, 3], gts [4, 8192, 3] (fp32).
  P[b,n,m] = ||gts[b,n] - preds[b,m]||^2
  loss = sum_b,m min_n P / 8192  +  sum_b,n min_m P / 8192

Sharding: 8 cores = 4 batches x 2 halves of N (the gts axis).
Core c handles b = c//2, n in [h*4096, (h+1)*4096), h = c%2, and all 8192 m.

Device kernel (SPMD, same program all cores):
  The distance matrix tile P[n_tile, m_chunk] is produced directly by the
  TensorEngine via an augmented contraction with fp16 hi/lo splitting
  (error-free fp16 products; only the |lo*lo| ~ 2^-22 cross term is dropped):
    per coord d: lhs rows (-2xh_d, -2xh_d, -2xl_d) vs rhs rows (yh_d, yl_d, yh_d)
    plus norm rows: (rxh,1), (rxl,1), (1,ryh), (1,ryl)       -> K = 13
  so P = lhsT.T @ rhs lands in PSUM (fp32) at 1 PE cycle/row.
  ScalarE copies each P chunk to SBUF as fp16; VectorE keeps two running mins:
    - min over n (partition axis, across n-tiles): tensor_tensor min into
      acc1[128, 8192] fp16 in 2x mode; collapsed across the 128 partitions
      at the end via PE transpose + free-dim reduce.
    - min over m (free axis): one pairwise fp16 2x min level, then a 1x
      tensor_reduce; per-n-tile partials reduced again across chunks.
Host: combine the two n-halves' partial min-over-n, then the two sums.
"""

import numpy as np

import concourse.bacc as bacc
import concourse.bass as bass
import concourse.mybir as mybir
import concourse.tile as tile
from concourse.bass_utils import run_bass_kernel_spmd

F32 = mybir.dt.float32
F16 = mybir.dt.float16

B = 4
N = 8192          # gts points per batch
M = 8192          # preds points per batch
HALF = N // 2     # n-range per core
NT = HALF // 128  # 32 n-tiles of 128
MCHUNK = 2048     # m-chunk (4 PSUM banks)
MC = M // MCHUNK  # 4 m-chunks
MMF = 512         # matmul moving free dim (1 PSUM bank of fp32 out)
QPC = MCHUNK // MMF  # 4 matmuls per chunk
K = 13            # augmented contraction dim (fp16 hi/lo split)
BIG = 60000.0     # running-min init (fits fp16)


def _main_loop(nc, tc, xs, ys, acc1, acc2, work_pool, chunk_pool, psum_pool):
    for i in range(NT):
        lhsT = xs[:, i * 128:(i + 1) * 128]
        # whole row of P for this n-tile, copied chunkwise to fp16 SBUF
        ct = chunk_pool.tile([128, M], F16, tag="ct", bufs=2)
        for j in range(MC):
            pt = psum_pool.tile([128, MCHUNK], F32, tag="pt")
            for q in range(QPC):
                nc.tensor.matmul(
                    pt[:, q * MMF:(q + 1) * MMF],
                    lhsT,
                    ys[:, j * MCHUNK + q * MMF: j * MCHUNK + (q + 1) * MMF],
                    start=True,
                    stop=True,
                )
            # downcast copy PSUM -> SBUF fp16 (ScalarE)
            nc.scalar.copy(ct[:, j * MCHUNK:(j + 1) * MCHUNK], pt[:])
        # elementwise running min over n-tiles (fp16 2x), whole row
        nc.vector.tensor_tensor(
            out=acc1[:], in0=ct[:], in1=acc1[:], op=mybir.AluOpType.min
        )
        # min over m for this n-tile: pairwise fp16 2x tree, then 1x reduce
        h = ct
        w = M
        while w > 512:
            w //= 2
            hn = chunk_pool.tile([128, w], F16, tag=f"h{w}", bufs=2)
            nc.vector.tensor_tensor(
                out=hn[:], in0=h[:, :w], in1=h[:, w:2 * w],
                op=mybir.AluOpType.min,
            )
            h = hn
        nc.vector.tensor_reduce(
            out=acc2[:, i:i + 1], in_=h[:],
            axis=mybir.AxisListType.X, op=mybir.AluOpType.min,
        )


def build_bass(reps=1):
    nc = bacc.Bacc()
    xa = nc.declare_dram_parameter("xa", [K, HALF], F16, isOutput=False)
    ya = nc.declare_dram_parameter("ya", [K, M], F16, isOutput=False)
    idh = nc.declare_dram_parameter("idh", [128, 128], F16, isOutput=False)
    idf = nc.declare_dram_parameter("idf", [128, 128], F32, isOutput=False)
    m1 = nc.declare_dram_parameter("m1", [M], F32, isOutput=True)
    m2 = nc.declare_dram_parameter("m2", [HALF], F32, isOutput=True)

    with tile.TileContext(nc) as tc:
        with (
            tc.tile_pool(name="const", bufs=1) as const_pool,
            tc.tile_pool(name="work", bufs=1) as work_pool,
            tc.tile_pool(name="chunk", bufs=3) as chunk_pool,
            tc.tile_pool(name="psum", bufs=2, space="PSUM") as psum_pool,
        ):
            xs = const_pool.tile([K, HALF], F16)
            ys = const_pool.tile([K, M], F16)
            idnh = const_pool.tile([128, 128], F16)
            idnf = const_pool.tile([128, 128], F32)
            nc.sync.dma_start(xs[:], xa[:])
            nc.sync.dma_start(ys[:], ya[:])
            nc.sync.dma_start(idnh[:], idh[:])
            nc.sync.dma_start(idnf[:], idf[:])

            # running min over n for every m, [partition=n%128, m]
            acc1 = work_pool.tile([128, M], F16)
            nc.gpsimd.memset(acc1[:], BIG)
            # per-n row mins (min over m), column i = n-tile i
            acc2 = work_pool.tile([128, NT], F32)

            import contextlib
            rep_ctx = (tc.For_i(0, reps, 1, name="timing")
                       if reps > 1 else contextlib.nullcontext())
            with rep_ctx:
                _main_loop(nc, tc, xs, ys, acc1, acc2, work_pool, chunk_pool,
                           psum_pool)
            # collapse acc1 across partitions: per 128-col block, transpose on
            # PE then free-dim min-reduce -> m1cols[p, c] = min_n P[n, c*128+p]
            m1cols = work_pool.tile([128, M // 128], F32)
            for c in range(M // 128):
                tr = psum_pool.tile([128, 128], F16, tag="pt")
                nc.tensor.transpose(tr[:], acc1[:, c * 128:(c + 1) * 128], idnh[:])
                nc.vector.tensor_reduce(
                    out=m1cols[:, c:c + 1], in_=tr[:],
                    axis=mybir.AxisListType.X, op=mybir.AluOpType.min,
                )

            # transpose [128, M/128] -> [M/128, 128] so DRAM store is contiguous
            trm1 = psum_pool.tile([128, 128], F32, tag="pt")
            nc.tensor.transpose(trm1[:M // 128, :], m1cols[:], idnf[:])
            m1row = work_pool.tile([M // 128, 128], F32)
            nc.scalar.copy(m1row[:], trm1[:M // 128, :])
            nc.sync.dma_start(m1.rearrange("(c p) -> c p", p=128), m1row[:])

            # same for acc2 [128, NT] -> [NT, 128]; n = i*128 + p
            trm2 = psum_pool.tile([128, 128], F32, tag="pt")
            nc.tensor.transpose(trm2[:NT, :], acc2[:], idnf[:])
            m2row = work_pool.tile([NT, 128], F32)
            nc.scalar.copy(m2row[:], trm2[:NT, :])
            nc.sync.dma_start(m2.rearrange("(i p) -> i p", p=128), m2row[:])

    nc.compile()
    return nc


def _split16(a):
    """fp32 array -> (hi, lo) fp16 with hi + lo ~= a."""
    hi = a.astype(np.float16)
    lo = (a - hi.astype(np.float32)).astype(np.float16)
    return hi, lo


def _augment(x, y):
    """x [HALF,3] gts half, y [M,3] preds -> (xa [K,HALF], ya [K,M]) fp16."""
    x = np.asarray(x, dtype=np.float32)
    y = np.asarray(y, dtype=np.float32)
    rx = (x * x).sum(axis=1)
    ry = (y * y).sum(axis=1)
    xh, xl = _split16(-2.0 * x)
    yh, yl = _split16(y)
    rxh, rxl = _split16(rx)
    ryh, ryl = _split16(ry)
    one_x = np.ones(x.shape[0], dtype=np.float16)
    one_y = np.ones(y.shape[0], dtype=np.float16)
    xa_rows = []
    ya_rows = []
    for d in range(3):
        xa_rows += [xh[:, d], xh[:, d], xl[:, d]]
        ya_rows += [yh[:, d], yl[:, d], yh[:, d]]
    xa_rows += [rxh, rxl, one_x, one_x]
    ya_rows += [one_y, one_y, ryh, ryl]
    xa = np.ascontiguousarray(np.stack(xa_rows))
    ya = np.ascontiguousarray(np.stack(ya_rows))
    return xa, ya


def run(preds, gts, reps=1):
    preds = np.ascontiguousarray(np.asarray(preds, dtype=np.float32))
    gts = np.ascontiguousarray(np.asarray(gts, dtype=np.float32))
    assert preds.shape == (B, M, 3) and gts.shape == (B, N, 3)

    nc = build_bass(reps=reps)
    idh = np.eye(128, dtype=np.float16)
    idf = np.eye(128, dtype=np.float32)
    in_maps = []
    for c in range(8):
        b, h = divmod(c, 2)
        xa, ya = _augment(gts[b, h * HALF:(h + 1) * HALF], preds[b])
        in_maps.append({"xa": xa, "ya": ya, "idh": idh, "idf": idf})

    res = run_bass_kernel_spmd(nc, in_maps, core_ids=list(range(8)))

    l1 = np.float64(0.0)
    l2 = np.float64(0.0)
    for b in range(B):
        p1 = np.minimum(res.results[2 * b]["m1"], res.results[2 * b + 1]["m1"])
        l1 += np.float64(p1.sum(dtype=np.float64))
        l2 += np.float64(res.results[2 * b]["m2"].sum(dtype=np.float64))
        l2 += np.float64(res.results[2 * b + 1]["m2"].sum(dtype=np.float64))
    loss = np.float32(l1 / M + l2 / N)
    return loss, res


def kernel(preds, gts):
    loss, _ = run(preds, gts)
    return np.asarray(loss, dtype=np.float32)


# revision 20
# speedup vs baseline: 1.2195x; 1.2195x over previous
"""Chamfer distance kernel for 8 Trainium2 NeuronCores.

Problem: preds [4, 8192, 3], gts [4, 8192, 3] (fp32).
  P[b,n,m] = ||gts[b,n] - preds[b,m]||^2
  loss = sum_b,m min_n P / 8192  +  sum_b,n min_m P / 8192

Sharding: 8 cores = 4 batches x 2 halves of N (the gts axis).
Core c handles b = c//2, n in [h*4096, (h+1)*4096), h = c%2, and all 8192 m.

Device kernel (SPMD, same program all cores):
  The distance matrix tile P[n_tile, m_chunk] is produced directly by the
  TensorEngine via an augmented contraction with fp16 hi/lo splitting
  (error-free fp16 products; only the |lo*lo| ~ 2^-22 cross term is dropped):
    per coord d: lhs rows (-2xh_d, -2xh_d, -2xl_d) vs rhs rows (yh_d, yl_d, yh_d)
    plus norm rows: (rxh,1), (rxl,1), (1,ryh), (1,ryl)       -> K = 13
  so P = lhsT.T @ rhs lands in PSUM (fp32) at 1 PE cycle/row.
  ScalarE copies each P chunk to SBUF as fp16; VectorE keeps two running mins:
    - min over n (partition axis, across n-tiles): tensor_tensor min into
      acc1[128, 8192] fp16 in 2x mode; collapsed across the 128 partitions
      at the end via PE transpose + free-dim reduce.
    - min over m (free axis): one pairwise fp16 2x min level, then a 1x
      tensor_reduce; per-n-tile partials reduced again across chunks.
Host: combine the two n-halves' partial min-over-n, then the two sums.
"""

import numpy as np

import concourse.bacc as bacc
import concourse.bass as bass
import concourse.mybir as mybir
import concourse.tile as tile
from concourse.bass_utils import run_bass_kernel_spmd

F32 = mybir.dt.float32
F16 = mybir.dt.float16

B = 4
N = 8192          # gts points per batch
M = 8192          # preds points per batch
HALF = N // 2     # n-range per core
NT = HALF // 128  # 32 n-tiles of 128
MCHUNK = 2048     # m-chunk (4 PSUM banks)
MC = M // MCHUNK  # 4 m-chunks
MMF = 512         # matmul moving free dim (1 PSUM bank of fp32 out)
QPC = MCHUNK // MMF  # 4 matmuls per chunk
K = 13            # augmented contraction dim (fp16 hi/lo split)
BIG = 60000.0     # running-min init (fits fp16)


def _main_loop(nc, tc, xs, ys, acc1, acc2, work_pool, chunk_pool, psum_pool):
    for i in range(NT):
        lhsT = xs[:, i * 128:(i + 1) * 128]
        # whole row of P for this n-tile, copied chunkwise to fp16 SBUF
        ct = chunk_pool.tile([128, M], F16, tag="ct", bufs=2)
        for j in range(MC):
            pt = psum_pool.tile([128, MCHUNK], F32, tag="pt")
            for q in range(QPC):
                nc.tensor.matmul(
                    pt[:, q * MMF:(q + 1) * MMF],
                    lhsT,
                    ys[:, j * MCHUNK + q * MMF: j * MCHUNK + (q + 1) * MMF],
                    start=True,
                    stop=True,
                )
            # downcast copy PSUM -> SBUF fp16 (ScalarE)
            nc.scalar.copy(ct[:, j * MCHUNK:(j + 1) * MCHUNK], pt[:])
        # elementwise running min over n-tiles (fp16 2x), whole row
        nc.vector.tensor_tensor(
            out=acc1[:], in0=ct[:], in1=acc1[:], op=mybir.AluOpType.min
        )
        # min over m for this n-tile: pairwise fp16 2x tree, then 1x reduce
        h = ct
        w = M
        while w > 256:
            w //= 2
            hn = chunk_pool.tile([128, w], F16, tag=f"h{w}", bufs=2)
            nc.vector.tensor_tensor(
                out=hn[:], in0=h[:, :w], in1=h[:, w:2 * w],
                op=mybir.AluOpType.min,
            )
            h = hn
        nc.vector.tensor_reduce(
            out=acc2[:, i:i + 1], in_=h[:],
            axis=mybir.AxisListType.X, op=mybir.AluOpType.min,
        )


def build_bass(reps=1):
    nc = bacc.Bacc()
    xa = nc.declare_dram_parameter("xa", [K, HALF], F16, isOutput=False)
    ya = nc.declare_dram_parameter("ya", [K, M], F16, isOutput=False)
    idh = nc.declare_dram_parameter("idh", [128, 128], F16, isOutput=False)
    idf = nc.declare_dram_parameter("idf", [128, 128], F32, isOutput=False)
    m1 = nc.declare_dram_parameter("m1", [M], F32, isOutput=True)
    m2 = nc.declare_dram_parameter("m2", [HALF], F32, isOutput=True)

    with tile.TileContext(nc) as tc:
        with (
            tc.tile_pool(name="const", bufs=1) as const_pool,
            tc.tile_pool(name="work", bufs=1) as work_pool,
            tc.tile_pool(name="chunk", bufs=3) as chunk_pool,
            tc.tile_pool(name="psum", bufs=2, space="PSUM") as psum_pool,
        ):
            xs = const_pool.tile([K, HALF], F16)
            ys = const_pool.tile([K, M], F16)
            idnh = const_pool.tile([128, 128], F16)
            idnf = const_pool.tile([128, 128], F32)
            nc.sync.dma_start(xs[:], xa[:])
            nc.sync.dma_start(ys[:], ya[:])
            nc.sync.dma_start(idnh[:], idh[:])
            nc.sync.dma_start(idnf[:], idf[:])

            # running min over n for every m, [partition=n%128, m]
            acc1 = work_pool.tile([128, M], F16)
            nc.gpsimd.memset(acc1[:], BIG)
            # per-n row mins (min over m), column i = n-tile i
            acc2 = work_pool.tile([128, NT], F32)

            import contextlib
            rep_ctx = (tc.For_i(0, reps, 1, name="timing")
                       if reps > 1 else contextlib.nullcontext())
            with rep_ctx:
                _main_loop(nc, tc, xs, ys, acc1, acc2, work_pool, chunk_pool,
                           psum_pool)
            # collapse acc1 across partitions: per 128-col block, transpose on
            # PE then free-dim min-reduce -> m1cols[p, c] = min_n P[n, c*128+p]
            # 4 transposed blocks share one PSUM tile; one 3D-AP reduce
            # ([128, 4, 128], axis=X) emits 4 block-mins at once.
            m1cols = work_pool.tile([128, M // 128], F32)
            for c in range(0, M // 128, 4):
                tr = psum_pool.tile([128, 512], F16, tag="pt")
                for q in range(4):
                    nc.tensor.transpose(
                        tr[:, q * 128:(q + 1) * 128],
                        acc1[:, (c + q) * 128:(c + q + 1) * 128], idnh[:],
                    )
                nc.vector.tensor_reduce(
                    out=m1cols[:, c:c + 4],
                    in_=tr.rearrange("p (c q) -> p c q", q=128),
                    axis=mybir.AxisListType.X, op=mybir.AluOpType.min,
                )

            # transpose [128, M/128] -> [M/128, 128] so DRAM store is contiguous
            trm1 = psum_pool.tile([128, 128], F32, tag="pt")
            nc.tensor.transpose(trm1[:M // 128, :], m1cols[:], idnf[:])
            m1row = work_pool.tile([M // 128, 128], F32)
            nc.scalar.copy(m1row[:], trm1[:M // 128, :])
            nc.sync.dma_start(m1.rearrange("(c p) -> c p", p=128), m1row[:])

            # same for acc2 [128, NT] -> [NT, 128]; n = i*128 + p
            trm2 = psum_pool.tile([128, 128], F32, tag="pt")
            nc.tensor.transpose(trm2[:NT, :], acc2[:], idnf[:])
            m2row = work_pool.tile([NT, 128], F32)
            nc.scalar.copy(m2row[:], trm2[:NT, :])
            nc.sync.dma_start(m2.rearrange("(i p) -> i p", p=128), m2row[:])

    nc.compile()
    return nc


def _split16(a):
    """fp32 array -> (hi, lo) fp16 with hi + lo ~= a."""
    hi = a.astype(np.float16)
    lo = (a - hi.astype(np.float32)).astype(np.float16)
    return hi, lo


def _augment(x, y):
    """x [HALF,3] gts half, y [M,3] preds -> (xa [K,HALF], ya [K,M]) fp16."""
    x = np.asarray(x, dtype=np.float32)
    y = np.asarray(y, dtype=np.float32)
    rx = (x * x).sum(axis=1)
    ry = (y * y).sum(axis=1)
    xh, xl = _split16(-2.0 * x)
    yh, yl = _split16(y)
    rxh, rxl = _split16(rx)
    ryh, ryl = _split16(ry)
    one_x = np.ones(x.shape[0], dtype=np.float16)
    one_y = np.ones(y.shape[0], dtype=np.float16)
    xa_rows = []
    ya_rows = []
    for d in range(3):
        xa_rows += [xh[:, d], xh[:, d], xl[:, d]]
        ya_rows += [yh[:, d], yl[:, d], yh[:, d]]
    xa_rows += [rxh, rxl, one_x, one_x]
    ya_rows += [one_y, one_y, ryh, ryl]
    xa = np.ascontiguousarray(np.stack(xa_rows))
    ya = np.ascontiguousarray(np.stack(ya_rows))
    return xa, ya


def run(preds, gts, reps=1):
    preds = np.ascontiguousarray(np.asarray(preds, dtype=np.float32))
    gts = np.ascontiguousarray(np.asarray(gts, dtype=np.float32))
    assert preds.shape == (B, M, 3) and gts.shape == (B, N, 3)

    nc = build_bass(reps=reps)
    idh = np.eye(128, dtype=np.float16)
    idf = np.eye(128, dtype=np.float32)
    in_maps = []
    for c in range(8):
        b, h = divmod(c, 2)
        xa, ya = _augment(gts[b, h * HALF:(h + 1) * HALF], preds[b])
        in_maps.append({"xa": xa, "ya": ya, "idh": idh, "idf": idf})

    res = run_bass_kernel_spmd(nc, in_maps, core_ids=list(range(8)))

    l1 = np.float64(0.0)
    l2 = np.float64(0.0)
    for b in range(B):
        p1 = np.minimum(res.results[2 * b]["m1"], res.results[2 * b + 1]["m1"])
        l1 += np.float64(p1.sum(dtype=np.float64))
        l2 += np.float64(res.results[2 * b]["m2"].sum(dtype=np.float64))
        l2 += np.float64(res.results[2 * b + 1]["m2"].sum(dtype=np.float64))
    loss = np.float32(l1 / M + l2 / N)
    return loss, res


def kernel(preds, gts):
    loss, _ = run(preds, gts)
    return np.asarray(loss, dtype=np.float32)
